# revision 2
# baseline (speedup 1.0000x reference)
"""Trainium2 Bass kernel for dual cross-attention (nn_CrossAttention).

Work split: 8 independent units = 4 batches x 2 attention branches; one unit
per NeuronCore. Per core: q_cat = [Wq1@x1; Wq2@x2] (16ch), K = Wk@x_kv (16ch),
V^T computed directly in [key, chan] layout, then flash-style attention:
  E^T[key, q] = K_chunk^T @ q_cat      (PE, contract=16, 4-way row packing)
  P^T = exp(E^T)                       (ScalarE, PSUM->SBUF)
  outT[c, q] += V^T_aug[key, c]^T @ P^T (PE, contract=128)
V^T is augmented with a ones column so row 64 of outT accumulates the softmax
denominator; normalization = reciprocal + broadcast matmul + multiply.
Residual add and store in [c, q] layout (no transposes anywhere).
"""

import sys

import numpy as np

for _p in ("/opt/trn_rl_repo",):
    if _p not in sys.path:
        sys.path.insert(0, _p)

import concourse.mybir as mybir
import concourse.tile as tile
from concourse import bacc, bass_utils

FP32 = mybir.dt.float32
B, C, HW = 4, 64, 4096
CQ = 16  # channels of concatenated query / key
CA = 65  # augmented contract dim (64 channels + ones row)
QG = 512  # query-group width (one PSUM bank of fp32)
NQG = HW // QG  # 8
KC = 128  # key-chunk width (matmul M / partition dim)
NKC = HW // KC  # 32

# Per query-group the 32 key chunks are processed in 9 groups sized
# 4,3,4,3,... so groups alternate between two PSUM regions (4 banks / 3 banks)
# letting the next group's energy matmuls overlap the current group's exp.
GROUPS = []
_k = 0
for _gi in range(9):
    _n = 4 if _gi % 2 == 0 else 3
    GROUPS.append(list(range(_k, _k + _n)))
    _k += _n
assert _k == NKC


def build_nc():
    nc = bacc.Bacc(None, target_bir_lowering=False)
    xq1 = nc.dram_tensor("xq1", [CA, HW], FP32, kind="ExternalInput")
    xq2 = nc.dram_tensor("xq2", [CA, HW], FP32, kind="ExternalInput")
    xkv = nc.dram_tensor("xkv", [CA, HW], FP32, kind="ExternalInput")
    wq1t = nc.dram_tensor("wq1t", [CA, 128], FP32, kind="ExternalInput")
    wq2t = nc.dram_tensor("wq2t", [CA, 128], FP32, kind="ExternalInput")
    wkt = nc.dram_tensor("wkt", [CA, 128], FP32, kind="ExternalInput")
    wvt1 = nc.dram_tensor("wvt1", [CA, CA], FP32, kind="ExternalInput")
    out = nc.dram_tensor("out", [C, HW], FP32, kind="ExternalOutput")

    with tile.TileContext(nc) as tc:
        with (
            tc.tile_pool(name="const", bufs=1) as cpool,
            tc.tile_pool(name="xin", bufs=1) as xpool,
            tc.tile_pool(name="proj", bufs=1) as prpool,
            tc.tile_pool(name="pt", bufs=2) as ptpool,
            tc.tile_pool(name="outp", bufs=2) as opool,
        ):
            xq1_sb = xpool.tile([CA, HW], FP32, tag="xq1")
            xq2_sb = xpool.tile([CA, HW], FP32, tag="xq2")
            xkv_sb = xpool.tile([CA, HW], FP32, tag="xkv")
            wq1t_sb = cpool.tile([CA, 128], FP32, tag="wq1t")
            wq2t_sb = cpool.tile([CA, 128], FP32, tag="wq2t")
            wkt_sb = cpool.tile([CA, 128], FP32, tag="wkt")
            wvt1_sb = cpool.tile([CA, CA], FP32, tag="wvt1")
            ones_sb = cpool.tile([1, C], FP32, tag="ones")
            q_rep = prpool.tile([128, HW], FP32, tag="qrep")
            k_rep = prpool.tile([128, HW], FP32, tag="krep")
            vt_sb = prpool.tile([128, NKC * CA], FP32, tag="vt")

            nc.gpsimd.memset(ones_sb[:], 1.0)
            nc.sync.dma_start(wq1t_sb[:], wq1t[:])
            nc.sync.dma_start(wq2t_sb[:], wq2t[:])
            nc.sync.dma_start(wkt_sb[:], wkt[:])
            nc.sync.dma_start(wvt1_sb[:], wvt1[:])
            for cc in range(NQG):
                sl = slice(cc * QG, (cc + 1) * QG)
                nc.sync.dma_start(xq1_sb[:, sl], xq1[:, sl])
                nc.sync.dma_start(xq2_sb[:, sl], xq2[:, sl])
                nc.sync.dma_start(xkv_sb[:, sl], xkv[:, sl])

            # Phase 1: projections. q_rep/k_rep hold q_cat/K replicated into
            # the four 32-partition row groups (via the zero-padded replicated
            # weight layout prepared on host); vt holds V^T chunks [128, 65].
            with tc.tile_pool(name="psum1", bufs=2, space="PSUM") as pp1:
                for cc in range(NQG):
                    sl = slice(cc * QG, (cc + 1) * QG)
                    pq = pp1.tile([128, QG], FP32, tag="pq")
                    nc.tensor.matmul(pq[:], wq1t_sb[:], xq1_sb[:, sl], start=True, stop=False)
                    nc.tensor.matmul(pq[:], wq2t_sb[:], xq2_sb[:, sl], start=False, stop=True)
                    nc.vector.tensor_copy(q_rep[:, sl], pq[:])
                    pk = pp1.tile([128, QG], FP32, tag="pk")
                    nc.tensor.matmul(pk[:], wkt_sb[:], xkv_sb[:, sl], start=True, stop=True)
                    nc.vector.tensor_copy(k_rep[:, sl], pk[:])
                    for j in range(4):
                        kk = cc * 4 + j
                        pv = pp1.tile([128, CA], FP32, tag="pv")
                        nc.tensor.matmul(
                            pv[:], xkv_sb[:, kk * KC:(kk + 1) * KC], wvt1_sb[:],
                            start=True, stop=True,
                        )
                        nc.vector.tensor_copy(vt_sb[:, kk * CA:(kk + 1) * CA], pv[:])

            # Phase 2: attention.
            with (
                tc.tile_pool(name="psA", bufs=1, space="PSUM") as ppa,
                tc.tile_pool(name="psB", bufs=1, space="PSUM") as ppb,
                tc.tile_pool(name="psO", bufs=1, space="PSUM") as ppo,
            ):
                for qg in range(NQG):
                    qsl = slice(qg * QG, (qg + 1) * QG)
                    po = ppo.tile([128, QG], FP32, tag="po")
                    for kcs in GROUPS:
                        glen = len(kcs)
                        if glen == 4:
                            pe_t = ppa.tile([128, 4 * QG], FP32, tag="pea")
                            pt_t = ptpool.tile([128, 4 * QG], FP32, tag="pta")
                        else:
                            pe_t = ppb.tile([128, 3 * QG], FP32, tag="peb")
                            pt_t = ptpool.tile([128, 3 * QG], FP32, tag="ptb")
                        for i, kc in enumerate(kcs):
                            rb = 32 * i
                            nc.tensor.matmul(
                                pe_t[:, i * QG:(i + 1) * QG],
                                k_rep[rb:rb + CQ, kc * KC:(kc + 1) * KC],
                                q_rep[rb:rb + CQ, qsl],
                                start=True, stop=True,
                                tile_position=(rb, 0),
                            )
                        nc.scalar.activation(
                            pt_t[:], pe_t[:], mybir.ActivationFunctionType.Exp
                        )
                        for i, kc in enumerate(kcs):
                            nc.tensor.matmul(
                                po[0:CA, :],
                                vt_sb[:, kc * CA:(kc + 1) * CA],
                                pt_t[:, i * QG:(i + 1) * QG],
                                start=(kc == 0), stop=(kc == NKC - 1),
                                skip_group_check=True,
                            )
                    # Normalize: row 64 of po is the softmax denominator.
                    rs = opool.tile([1, QG], FP32, tag="rs")
                    nc.vector.reciprocal(rs[:], po[C:C + 1, :])
                    nc.tensor.matmul(
                        po[C:C + C, :], ones_sb[:], rs[:],
                        start=True, stop=True, skip_group_check=True,
                    )
                    bc = opool.tile([C, QG], FP32, tag="bc")
                    nc.vector.tensor_copy(bc[:], po[C:C + C, :])
                    tmp = opool.tile([C, QG], FP32, tag="tmp")
                    nc.vector.tensor_mul(tmp[:], po[0:C, :], bc[:])
                    ot = opool.tile([C, QG], FP32, tag="ot")
                    nc.vector.tensor_add(ot[:], tmp[:], xkv_sb[0:C, qsl])
                    nc.sync.dma_start(out[:, qsl], ot[:])
    nc.finalize()
    return nc


_CACHE = {}


def _get_nc():
    if "nc" not in _CACHE:
        _CACHE["nc"] = build_nc()
    return _CACHE["nc"]


def _prep_in_maps(inputs):
    f32 = np.float32
    g = {k: np.asarray(v, f32) for k, v in inputs.items()}
    x1 = g["input1"].reshape(B, C, HW)
    x2 = g["input2"].reshape(B, C, HW)
    ones = np.ones((1, HW), f32)

    def wq_rep(Wq, bq, off):
        w = np.zeros((CA, 128), f32)
        for r in range(4):
            w[0:C, 32 * r + off:32 * r + off + 8] = Wq.T
            w[C, 32 * r + off:32 * r + off + 8] = bq
        return w

    def wk_rep(Wk, bk):
        w = np.zeros((CA, 128), f32)
        for r in range(4):
            w[0:C, 32 * r:32 * r + CQ] = Wk.T
            w[C, 32 * r:32 * r + CQ] = bk
        return w

    def wv_aug(Wv, bv):
        w = np.zeros((CA, CA), f32)
        w[0:C, 0:C] = Wv.T
        w[C, 0:C] = bv
        w[C, C] = 1.0
        return w

    wq1t = wq_rep(g["Wq1"], g["bq1"], 0)
    wq2t = wq_rep(g["Wq2"], g["bq2"], 8)
    wkt = [wk_rep(g["Wk1"], g["bk1"]), wk_rep(g["Wk2"], g["bk2"])]
    wvt1 = [wv_aug(g["Wv1"], g["bv1"]), wv_aug(g["Wv2"], g["bv2"])]

    in_maps = []
    for b in range(B):
        xq1 = np.ascontiguousarray(np.concatenate([x1[b], ones], axis=0))
        xq2 = np.ascontiguousarray(np.concatenate([x2[b], ones], axis=0))
        for which in range(2):
            in_maps.append({
                "xq1": xq1,
                "xq2": xq2,
                "xkv": xq1 if which == 0 else xq2,
                "wq1t": wq1t,
                "wq2t": wq2t,
                "wkt": wkt[which],
                "wvt1": wvt1[which],
            })
    return in_maps


def kernel(**inputs):
    in_maps = _prep_in_maps(inputs)
    res = bass_utils.run_bass_kernel_spmd(
        _get_nc(), in_maps, core_ids=list(range(8))
    )
    outs = [r["out"] for r in res.results]
    out1 = np.stack([outs[2 * b] for b in range(B)]).reshape(B, C, 64, 64)
    out2 = np.stack([outs[2 * b + 1] for b in range(B)]).reshape(B, C, 64, 64)
    return out1, out2
